# revision 15
# baseline (speedup 1.0000x reference)
"""Trainium2 Bass kernel for LoRA linear: y = x @ (W + 2*B@A).T + b.

Full inputs: x (8, 2048, 2048) f32, W (2048, 2048) f32, b (2048,) f32,
B (2048, 16) f32, A (16, 2048) f32.  Output (8, 2048, 2048) f32.

Sharding: data-parallel over the batch dim — core i computes
y[i] = x[i] @ w.T + b with the merged weight w = W + 2*B@A.

Per-core kernel (bf16 TensorEngine compute, f32 accumulate):
  phase 0 (all on the gpsimd software-DGE queue, ~12us cold start —
           HWDGE dispatch->data latency is ~7us anyway, and only
           gpsimd can cast): A, B staged bf16, x0, bias broadcast,
           then x1..; 2*B.T via PE transposes.
  phase W: W row-blocks f32 preloaded on both HWDGE queues (evens on
           scalar, odds on sync); ScalarE pre-cast w16c = bf16(wrow)
           releases the load buffer within ~2us so the queues stream
           at full rate instead of being paced by the PE; rank-16
           delta matmul in f32 PSUM; DVE merge w16 = w16c + delta;
           16 PE transposes of the merged rows, DVE evicts into wT.
  phase x (interleaved): per 128-row x tile: cast-DMA on gpsimd,
           16 PE transposes, ScalarE evicts.
  main:    per (row tile, 512-col bank): 16 bf16 matmuls into PSUM,
           VectorE adds the bias during eviction, per-bank stores on
           the sync queue (short tail, small ys tiles).
"""

import numpy as np

import concourse.bacc as bacc
import concourse.mybir as mybir
import concourse.tile as tile
from concourse import masks
from concourse.bass_utils import run_bass_kernel_spmd

N_CORES = 8
BATCH, S, D = 8, 2048, 2048
RANK = 16
SCALE = 2.0  # alpha / rank = 32 / 16
P = 128  # partitions
FREE = 512  # f32 elems per PSUM bank
ND = D // P  # 16 contraction tiles
NS = S // P  # 16 row tiles per core
NO = D // FREE  # 4 output banks per row tile
NG = ND // 4  # 4 groups of 4

F32 = mybir.dt.float32
BF16 = mybir.dt.bfloat16


def build_nc():
    nc = bacc.Bacc(
        "TRN2", target_bir_lowering=False, debug=False, num_devices=N_CORES
    )
    x_d = nc.dram_tensor("x", [S, D], F32, kind="ExternalInput").ap()
    W_d = nc.dram_tensor("W", [D, D], F32, kind="ExternalInput").ap()
    b_d = nc.dram_tensor("b", [D], F32, kind="ExternalInput").ap()
    B_d = nc.dram_tensor("B", [D, RANK], F32, kind="ExternalInput").ap()
    A_d = nc.dram_tensor("A", [RANK, D], F32, kind="ExternalInput").ap()
    out_d = nc.dram_tensor("out", [S, D], F32, kind="ExternalOutput").ap()

    with tile.TileContext(nc) as tc:
        with (
            tc.tile_pool(name="singles", bufs=1) as singles,
            tc.tile_pool(name="wt", bufs=1) as wtp,
        ):
            ident = singles.tile([P, P], BF16)
            masks.make_identity(nc, ident[:])

            A_sb = singles.tile([RANK, D], BF16)
            nc.gpsimd.dma_start(out=A_sb[:], in_=A_d[:])

            B2T = singles.tile([RANK, D], BF16)
            Bs = singles.tile([P, ND * RANK], BF16)
            nc.gpsimd.dma_start(
                out=Bs[:], in_=B_d.rearrange("(t p) r -> p t r", p=P)
            )

            bb = singles.tile([P, D], BF16)

            # merged transposed weight, bf16: wT[p, dt, o] = w[o, dt*128+p]
            wT = wtp.tile([P, ND, D], BF16)

            with (
                tc.tile_pool(name="wrow", bufs=4) as wrowp,
                tc.tile_pool(name="w16c", bufs=6) as w16cp,
                tc.tile_pool(name="w16", bufs=3) as w16p,
                tc.tile_pool(name="xstage", bufs=4) as xstage,
                tc.tile_pool(name="xTp", bufs=6) as xTp,
                tc.tile_pool(name="yout", bufs=8) as youtp,
                tc.tile_pool(name="dpsum", bufs=3, space="PSUM") as dpsum,
                tc.tile_pool(name="tpsum", bufs=3, space="PSUM") as tpsum,
                tc.tile_pool(name="gpsum", bufs=2, space="PSUM") as gpsum,
            ):
                def x_load(st):
                    xs = xstage.tile([P, D], BF16, tag="xs")
                    nc.gpsimd.dma_start(
                        out=xs[:], in_=x_d[st * P : (st + 1) * P, :]
                    )
                    return xs

                # gpsimd queue order: A, B, x0, bias, x1, x2, ...
                xs0 = x_load(0)
                nc.gpsimd.dma_start(
                    out=bb[:], in_=b_d[None, :].broadcast_to([P, D])
                )
                xs12 = [x_load(1), x_load(2)]

                # W row-blocks preloaded, evens on scalar / odds on sync
                wrows = []
                for ot in range(ND):
                    wrow = wrowp.tile([P, D], F32, tag="wrow")
                    eng = nc.scalar if ot % 2 == 0 else nc.sync
                    eng.dma_start(
                        out=wrow[:], in_=W_d[ot * P : (ot + 1) * P, :]
                    )
                    wrows.append(wrow)

                # ScalarE pre-cast: releases wrow quickly so the HWDGE
                # queues stream W at full rate instead of waiting on the
                # PE-paced delta/merge chain
                w16cs = []
                for ot in range(ND):
                    w16c = w16cp.tile([P, D], BF16, tag="w16c")
                    nc.scalar.copy(w16c[:], wrows[ot][:])
                    w16cs.append(w16c)

                # 2*B.T from the staged B tiles (PE transposes, tiny)
                for g in range(NG):
                    bps = tpsum.tile([RANK, 4 * P], BF16, tag="tp")
                    for j in range(4):
                        t = 4 * g + j
                        nc.tensor.matmul(
                            bps[:, j * P : (j + 1) * P],
                            Bs[:, t * RANK : (t + 1) * RANK],
                            ident[:],
                            is_transpose=True,
                            start=(j == 0),
                            stop=(j == 3),
                        )
                    nc.vector.tensor_scalar_mul(
                        B2T[:, g * 4 * P : (g + 1) * 4 * P], bps[:], SCALE
                    )

                def x_transpose(xs):
                    xT = xTp.tile([P, ND, P], BF16, tag="xT")
                    for g in range(2):
                        tp = tpsum.tile([P, 8 * P], BF16, tag="tp")
                        for j in range(8):
                            dt = 8 * g + j
                            nc.tensor.matmul(
                                tp[:, j * P : (j + 1) * P],
                                xs[:, dt * P : (dt + 1) * P],
                                ident[:],
                                is_transpose=True,
                                start=(j == 0),
                                stop=(j == 7),
                            )
                        nc.scalar.copy(xT[:, 8 * g : 8 * (g + 1), :], tp[:])
                    return xT

                xTs = [x_transpose(xs0)]

                # ---- merged-weight build ----
                def w_compute(ot):
                    w16 = w16p.tile([P, D], BF16, tag="w16")
                    dps = [
                        dpsum.tile([P, FREE], F32, tag="dp", name=f"dp{ot}_{g}")
                        for g in range(NG)
                    ]
                    for g in range(NG):
                        nc.tensor.matmul(
                            dps[g][:],
                            B2T[:, ot * P : (ot + 1) * P],
                            A_sb[:, g * FREE : (g + 1) * FREE],
                            start=True,
                            stop=True,
                        )
                    for g in range(NG):
                        nc.vector.tensor_add(
                            w16[:, g * FREE : (g + 1) * FREE],
                            dps[g][:],
                            w16cs[ot][:, g * FREE : (g + 1) * FREE],
                        )
                    for g in range(2):
                        tp = tpsum.tile([P, 8 * P], BF16, tag="tp")
                        for j in range(8):
                            dt = 8 * g + j
                            nc.tensor.matmul(
                                tp[:, j * P : (j + 1) * P],
                                w16[:, dt * P : (dt + 1) * P],
                                ident[:],
                                is_transpose=True,
                                start=(j == 0),
                                stop=(j == 7),
                            )
                        nc.vector.tensor_scalar_mul(
                            wT[:, 8 * g : 8 * (g + 1), ot * P : (ot + 1) * P],
                            tp[:],
                            1.0,
                        )

                def x_chain(st):
                    return x_transpose(x_load(st))

                xTs.append(x_transpose(xs12[0]))
                xTs.append(x_transpose(xs12[1]))

                for ot in range(ND):
                    w_compute(ot)
                    if ot in (5, 9, 13):
                        xTs.append(x_chain(len(xTs)))
                PRE = len(xTs)  # 6

                # ---- main loop: y = x @ wT + b ----
                for st in range(NS):
                    if st + PRE < NS:
                        xTs.append(x_chain(st + PRE))
                    xT = xTs[st]
                    for oc in range(NO):
                        gp = gpsum.tile([P, FREE], F32)
                        for dt in range(ND):
                            nc.tensor.matmul(
                                gp[:],
                                xT[:, dt, :],
                                wT[:, dt, oc * FREE : (oc + 1) * FREE],
                                start=(dt == 0),
                                stop=(dt == ND - 1),
                            )
                        ys = youtp.tile([P, FREE], F32, tag="ys")
                        nc.vector.tensor_add(
                            ys[:], gp[:], bb[:, oc * FREE : (oc + 1) * FREE]
                        )
                        nc.sync.dma_start(
                            out=out_d[
                                st * P : (st + 1) * P,
                                oc * FREE : (oc + 1) * FREE,
                            ],
                            in_=ys[:],
                        )

    nc.compile()
    return nc


_NC_CACHE = None


def _get_nc():
    global _NC_CACHE
    if _NC_CACHE is None:
        _NC_CACHE = build_nc()
    return _NC_CACHE


def make_in_maps(x, W, b, B, A):
    x = np.ascontiguousarray(x, dtype=np.float32)
    W = np.ascontiguousarray(W, dtype=np.float32)
    b = np.ascontiguousarray(b, dtype=np.float32)
    B = np.ascontiguousarray(B, dtype=np.float32)
    A = np.ascontiguousarray(A, dtype=np.float32)
    return [
        {"x": x[i], "W": W, "b": b, "B": B, "A": A} for i in range(N_CORES)
    ]


def run(inputs, **spmd_kwargs):
    """Run the SPMD kernel; returns (output, BassKernelResults)."""
    nc = _get_nc()
    in_maps = make_in_maps(**inputs)
    res = run_bass_kernel_spmd(nc, in_maps, core_ids=list(range(N_CORES)), **spmd_kwargs)
    out = np.stack([res.results[i]["out"] for i in range(N_CORES)]).astype(np.float32)
    return out, res


def kernel(x, W, b, B, A):
    out, _ = run({"x": x, "W": W, "b": b, "B": B, "A": A})
    return out
